# revision 8
# baseline (speedup 1.0000x reference)
"""GraphVAE Trainium2 kernel: 2-layer GCN + VAE heads + dense per-graph decoder.

Sharding: data-parallel over graphs (32 graphs/core on 8 cores). Each graph
gets a fixed SLOT of node rows so all device-side slicing is static/SPMD.
GCN aggregation = indirect-DMA row gather + one-hot matmul scatter in PSUM.
Layer1->Layer2 handoff via chunked AllGather of G2 = H1 @ W2.
"""
import sys

for p in ("/opt/trn_rl_repo", "/root/.axon_site/_ro/trn_rl_repo"):
    if p not in sys.path:
        sys.path.insert(0, p)

import time as _time

import numpy as np
import ml_dtypes

import concourse.bass as bass
import concourse.bacc as bacc
import concourse.mybir as mybir
import concourse.tile as tile
from concourse import bass_utils

NCORES = 8
N = 50000
NUM_GRAPHS = 256
GPD = NUM_GRAPHS // NCORES  # graphs per device
HID = 128
LAT = 64
MAX_NODES = 320
SCALE = LAT ** -0.5
K_CH = 4  # collective chunks

BF16 = ml_dtypes.bfloat16

_CACHE = {}
LAST_EXEC_NS = None
LAST_META = None
TRACE = False


def _build_program(SLOT, C, dec_bias, sim=False):
    # C: per-tile chunk counts, tuple of length T_TILES (same on all cores)
    LOCAL = GPD * SLOT
    T_TILES = LOCAL // 128
    GLOBAL = NCORES * LOCAL
    CHOF = [0]
    for ct in C:
        CHOF.append(CHOF[-1] + ct)
    TC = CHOF[-1]
    CHROWS = LOCAL // K_CH
    TPCH = CHROWS // 128  # tiles per collective chunk

    nc = bacc.Bacc("TRN2", target_bir_lowering=False, debug=False,
                   enable_asserts=True, num_devices=(1 if sim else NCORES))
    dt = mybir.dt

    xT = nc.dram_tensor("xT", [128, GLOBAL], dt.bfloat16, kind="ExternalInput")
    epsT = nc.dram_tensor("epsT", [LAT, LOCAL], dt.float32, kind="ExternalInput")
    srcidx = nc.dram_tensor("srcidx", [128, TC], dt.int32, kind="ExternalInput")
    dstm = nc.dram_tensor("dstm", [128, TC], dt.float32, kind="ExternalInput")
    normm = nc.dram_tensor("normm", [128, TC], dt.float32, kind="ExternalInput")
    w1 = nc.dram_tensor("w1", [HID, HID], dt.bfloat16, kind="ExternalInput")
    w2 = nc.dram_tensor("w2", [HID, HID], dt.bfloat16, kind="ExternalInput")
    wmulv = nc.dram_tensor("wmulv", [HID, HID], dt.bfloat16, kind="ExternalInput")
    b1d = nc.dram_tensor("b1d", [HID], dt.float32, kind="ExternalInput")
    b2d = nc.dram_tensor("b2d", [HID], dt.float32, kind="ExternalInput")
    bmulvd = nc.dram_tensor("bmulvd", [HID], dt.float32, kind="ExternalInput")
    iotad = nc.dram_tensor("iotad", [128, 128], dt.float32, kind="ExternalInput")

    mulv_out = nc.dram_tensor("mulv", [128, LOCAL], dt.float32, kind="ExternalOutput")
    adj_out = nc.dram_tensor("adj", [GPD, SLOT, SLOT], dt.float32, kind="ExternalOutput")

    g1t = nc.dram_tensor("g1t", [GLOBAL, HID], dt.bfloat16, kind="Internal")
    g2full = nc.dram_tensor("g2full", [GLOBAL, HID], dt.bfloat16,
                            kind="Internal", addr_space="Shared")

    AF = mybir.ActivationFunctionType
    OP = mybir.AluOpType

    with tile.TileContext(nc) as tc:
        with tc.tile_pool(name="persist", bufs=1) as pp, \
             tc.tile_pool(name="stream", bufs=3) as sp, \
             tc.tile_pool(name="gath", bufs=12) as gp, \
             tc.tile_pool(name="psA", bufs=6, space="PSUM") as psA, \
             tc.tile_pool(name="psB", bufs=2, space="PSUM") as psB, \
             tc.tile_pool(name="dram", bufs=1, space="DRAM") as dramp:

            # ---- persistent loads ----
            w1_t = pp.tile([HID, HID], dt.bfloat16, tag="w1")
            nc.sync.dma_start(out=w1_t[:], in_=w1[:])
            w2_t = pp.tile([HID, HID], dt.bfloat16, tag="w2")
            nc.sync.dma_start(out=w2_t[:], in_=w2[:])
            wmulv_t = pp.tile([HID, HID], dt.bfloat16, tag="wmulv")
            nc.sync.dma_start(out=wmulv_t[:], in_=wmulv[:])
            b1_t = pp.tile([HID, 1], dt.float32, tag="b1")
            nc.sync.dma_start(out=b1_t[:], in_=b1d[:, None])
            b2_t = pp.tile([HID, 1], dt.float32, tag="b2")
            nc.sync.dma_start(out=b2_t[:], in_=b2d[:, None])
            bmulv_t = pp.tile([HID, 1], dt.float32, tag="bmulv")
            nc.sync.dma_start(out=bmulv_t[:], in_=bmulvd[:, None])
            iota_t = pp.tile([128, 128], dt.float32, tag="iota")
            nc.sync.dma_start(out=iota_t[:], in_=iotad[:])
            srcidx_t = pp.tile([128, TC], dt.int32, tag="srcidx")
            nc.sync.dma_start(out=srcidx_t[:], in_=srcidx[:])
            dstm_t = pp.tile([128, TC], dt.float32, tag="dstm")
            nc.sync.dma_start(out=dstm_t[:], in_=dstm[:])
            normm_t = pp.tile([128, TC], dt.float32, tag="normm")
            nc.sync.dma_start(out=normm_t[:], in_=normm[:])
            epsT_t = pp.tile([LAT, LOCAL], dt.float32, tag="eps")
            nc.sync.dma_start(out=epsT_t[:], in_=epsT[:])
            decb_t = pp.tile([128, 1], dt.float32, tag="decb")
            nc.gpsimd.memset(decb_t[:], float(dec_bias))

            # ---- phase A: G1 = x @ W1 for ALL padded-global rows (replicated) ----
            g1t_v = g1t[:].rearrange("(n p) f -> n p f", p=128)  # [GLOBAL/128, 128, 128]
            n_sup = GLOBAL // 512
            for s in range(n_sup):
                xt_tile = sp.tile([128, 512], dt.bfloat16, tag="xt")
                nc.sync.dma_start(out=xt_tile[:], in_=xT[:, s * 512:(s + 1) * 512])
                ps = psB.tile([128, 512], dt.float32, tag="psB", space="PSUM")
                for t in range(4):
                    nc.tensor.matmul(out=ps[:, t * 128:(t + 1) * 128],
                                     lhsT=xt_tile[:, t * 128:(t + 1) * 128],
                                     rhs=w1_t[:], start=True, stop=True)
                g1_s = sp.tile([128, 512], dt.bfloat16, tag="g1s")
                nc.scalar.activation(out=g1_s[:], in_=ps[:], func=AF.Copy)
                for t in range(4):
                    nc.sync.dma_start(out=g1t_v[s * 4 + t],
                                      in_=g1_s[:, t * 128:(t + 1) * 128])

            # ---- aggregation helper ----
            h1T = pp.tile([128, LOCAL], dt.bfloat16, tag="h1T")
            h2T = pp.tile([128, LOCAL], dt.bfloat16, tag="h2T")

            def gcn_layer(src_table, bias_t, outT):
                for tl in range(T_TILES):
                    ps = psA.tile([128, 128], dt.float32, tag="psA", space="PSUM")
                    for c in range(C[tl]):
                        ch = CHOF[tl] + c
                        g = gp.tile([128, 128], dt.bfloat16, tag="g")
                        nc.gpsimd.indirect_dma_start(
                            out=g[:], out_offset=None, in_=src_table[:],
                            in_offset=bass.IndirectOffsetOnAxis(
                                ap=srcidx_t[:, ch:ch + 1], axis=0))
                        s_t = gp.tile([128, 128], dt.bfloat16, tag="s")
                        nc.vector.tensor_scalar(
                            out=s_t[:], in0=iota_t[:],
                            scalar1=dstm_t[:, ch:ch + 1],
                            scalar2=normm_t[:, ch:ch + 1],
                            op0=OP.is_equal, op1=OP.mult)
                        nc.tensor.matmul(out=ps[:], lhsT=g[:], rhs=s_t[:],
                                         start=(c == 0), stop=(c == C[tl] - 1))
                    nc.scalar.activation(out=outT[:, tl * 128:(tl + 1) * 128],
                                         in_=ps[:], func=AF.Relu, bias=bias_t[:])

            # ---- phase B: layer-1 aggregation -> H1T ----
            gcn_layer(g1t, b1_t, h1T)

            # ---- phase C: G2 = H1 @ W2 (shard) -> chunked AllGather ----
            ag_ins = []
            for k in range(K_CH):
                agk = dramp.tile([CHROWS, HID], dt.bfloat16, tag=f"agin{k}")
                ag_ins.append(agk)
            g2full_v = g2full[:].rearrange("(k d r) f -> k (d r) f",
                                           k=K_CH, d=NCORES)
            for k in range(K_CH):
                agk_v = ag_ins[k][:].rearrange("(t p) f -> t p f", p=128)
                for i in range(TPCH):
                    tl = k * TPCH + i
                    ps = psB.tile([128, 512], dt.float32, tag="psB", space="PSUM")
                    nc.tensor.matmul(out=ps[:, :128],
                                     lhsT=h1T[:, tl * 128:(tl + 1) * 128],
                                     rhs=w2_t[:], start=True, stop=True)
                    g2_s = sp.tile([128, 128], dt.bfloat16, tag="g2s")
                    nc.scalar.activation(out=g2_s[:], in_=ps[:, :128], func=AF.Copy)
                    nc.sync.dma_start(out=agk_v[i], in_=g2_s[:])
                if sim:
                    nc.sync.dma_start(out=g2full_v[k][:CHROWS], in_=ag_ins[k][:])
                else:
                    nc.gpsimd.collective_compute(
                        "AllGather", OP.bypass,
                        replica_groups=[list(range(NCORES))],
                        ins=[ag_ins[k].opt()],
                        outs=[g2full_v[k].opt()])

            # ---- phase D: layer-2 aggregation -> H2T ----
            gcn_layer(g2full, b2_t, h2T)

            # ---- phase E: heads mulvT = Wmulv^T @ H2T + b ----
            mulvT = pp.tile([128, LOCAL], dt.float32, tag="mulvT")
            for s in range(LOCAL // 512):
                ps = psB.tile([128, 512], dt.float32, tag="psB", space="PSUM")
                nc.tensor.matmul(out=ps[:], lhsT=wmulv_t[:],
                                 rhs=h2T[:, s * 512:(s + 1) * 512],
                                 start=True, stop=True)
                nc.scalar.activation(out=mulvT[:, s * 512:(s + 1) * 512],
                                     in_=ps[:], func=AF.Identity, bias=bmulv_t[:])
            nc.sync.dma_start(out=mulv_out[:], in_=mulvT[:])

            # ---- phase F: z = mu + eps * exp(0.5 * clip(lv)) ----
            std_t = pp.tile([LAT, LOCAL], dt.float32, tag="std")
            zT = pp.tile([LAT, LOCAL], dt.float32, tag="zT")
            nc.vector.tensor_scalar(out=std_t[:], in0=mulvT[LAT:, :],
                                    scalar1=20.0, scalar2=-20.0,
                                    op0=OP.min, op1=OP.max)
            nc.scalar.activation(out=std_t[:], in_=std_t[:], func=AF.Exp, scale=0.5)
            nc.vector.tensor_tensor(out=std_t[:], in0=std_t[:], in1=epsT_t[:],
                                    op=OP.mult)
            nc.vector.tensor_tensor(out=zT[:], in0=std_t[:], in1=mulvT[:LAT, :],
                                    op=OP.add)

            # ---- phase G: decoder adj = sigmoid(SCALE * z z^T + bias) per graph ----
            m_offs = []
            off = 0
            while off < SLOT:
                mh = min(128, SLOT - off)
                m_offs.append((off, mh))
                off += mh
            for gi in range(GPD):
                base = gi * SLOT
                for (off, mh) in m_offs:
                    ps = psB.tile([128, 512], dt.float32, tag="psB", space="PSUM")
                    nc.tensor.matmul(out=ps[:mh, :SLOT],
                                     lhsT=zT[:, base + off:base + off + mh],
                                     rhs=zT[:, base:base + SLOT],
                                     start=True, stop=True)
                    a_s = sp.tile([128, SLOT], dt.float32, tag="as")
                    nc.scalar.activation(out=a_s[:mh, :], in_=ps[:mh, :SLOT],
                                         func=AF.Sigmoid, scale=float(SCALE),
                                         bias=decb_t[:mh, :])
                    nc.sync.dma_start(out=adj_out[gi, off:off + mh, :],
                                      in_=a_s[:mh, :])

    nc.compile()
    return nc


def kernel(x, edge_index, batch, eps, W1, b1, W2, b2, Wmu, bmu, Wlv, blv, dec_bias):
    global LAST_EXEC_NS
    x = np.asarray(x, dtype=np.float32)
    edge_index = np.asarray(edge_index)
    batch = np.asarray(batch).astype(np.int64)
    eps = np.asarray(eps, dtype=np.float32)

    counts = np.bincount(batch, minlength=NUM_GRAPHS)
    maxcnt = int(counts.max())
    SLOT = 256 if maxcnt <= 256 else 320
    assert maxcnt <= SLOT, f"graph too big: {maxcnt}"
    LOCAL = GPD * SLOT
    T_TILES = LOCAL // 128
    GLOBAL = NCORES * LOCAL
    CHROWS = LOCAL // K_CH

    ptr = np.concatenate([[0], np.cumsum(counts)])
    pos = np.arange(N, dtype=np.int64) - ptr[batch]
    dev_of_node = batch // GPD
    local_row = (batch % GPD) * SLOT + pos
    kk = local_row // CHROWS
    gl = kk * (NCORES * CHROWS) + dev_of_node * CHROWS + (local_row % CHROWS)

    src = edge_index[0].astype(np.int64)
    dst = edge_index[1].astype(np.int64)
    loop = np.arange(N, dtype=np.int64)
    src_all = np.concatenate([src, loop])
    dst_all = np.concatenate([dst, loop])
    deg = np.bincount(dst_all, minlength=N).astype(np.float32)
    dinv = np.where(deg > 0, deg ** -0.5, 0.0).astype(np.float32)
    norm = dinv[src_all] * dinv[dst_all]

    dev_e = dev_of_node[dst_all]
    ldst = local_row[dst_all]
    tile_e = ldst // 128
    dstloc = (ldst % 128).astype(np.float32)
    gsrc = gl[src_all].astype(np.int32)

    key = (dev_e * T_TILES + tile_e).astype(np.int64)
    order = np.argsort(key, kind="stable")
    key_s = key[order]
    gsrc_s = gsrc[order]
    dstloc_s = dstloc[order]
    norm_s = norm[order]
    gcounts = np.bincount(key_s, minlength=NCORES * T_TILES)
    per_tile_max = gcounts.reshape(NCORES, T_TILES).max(axis=0)
    C = tuple(int(c) for c in np.maximum(1, np.ceil(per_tile_max / 128).astype(np.int64)))
    chof = np.concatenate([[0], np.cumsum(C)])
    TC = int(chof[-1])

    # position of each edge within its padded (device,tile) group
    gstart = np.concatenate([[0], np.cumsum(gcounts)])[:-1]
    within = np.arange(len(key_s), dtype=np.int64) - gstart[key_s]
    dev_of_e = key_s // T_TILES
    tile_of_e = key_s % T_TILES
    slotpos = (dev_of_e * TC + chof[tile_of_e]) * 128 + within

    src_pad = np.zeros(NCORES * TC * 128, dtype=np.int32)
    dst_pad = np.full(NCORES * TC * 128, -1.0, dtype=np.float32)
    norm_pad = np.zeros(NCORES * TC * 128, dtype=np.float32)
    src_pad[slotpos] = gsrc_s
    dst_pad[slotpos] = dstloc_s
    norm_pad[slotpos] = norm_s
    # reshape to per-device [TC, 128] -> [128, TC]
    src_pad = src_pad.reshape(NCORES, TC, 128).transpose(0, 2, 1).copy()
    dst_pad = dst_pad.reshape(NCORES, TC, 128).transpose(0, 2, 1).astype(np.float32)
    norm_pad = norm_pad.reshape(NCORES, TC, 128).transpose(0, 2, 1).astype(np.float32)

    xT_pad = np.zeros((128, GLOBAL), dtype=BF16)
    xT_pad[:, gl] = x.T.astype(BF16)
    epsT_all = np.zeros((NCORES, LAT, LOCAL), dtype=np.float32)
    epsT_all[dev_of_node, :, local_row] = eps  # fancy: [N, LAT] -> slots
    wmulv = np.concatenate([np.asarray(Wmu, np.float32),
                            np.asarray(Wlv, np.float32)], axis=1)
    bmulv = np.concatenate([np.asarray(bmu, np.float32), np.asarray(blv, np.float32)])
    iota_np = np.ascontiguousarray(np.broadcast_to(np.arange(128, dtype=np.float32), (128, 128)))

    global LAST_META
    LAST_META = dict(SLOT=SLOT, C=C, dec_bias=float(dec_bias))
    ck = (SLOT, C)  # C is a tuple -> hashable
    if ck not in _CACHE:
        _CACHE[ck] = _build_program(SLOT, C, float(dec_bias))
    nc = _CACHE[ck]

    in_maps = []
    for d in range(NCORES):
        in_maps.append({
            "xT": xT_pad,
            "epsT": np.ascontiguousarray(epsT_all[d]),
            "srcidx": np.ascontiguousarray(src_pad[d]),
            "dstm": np.ascontiguousarray(dst_pad[d]),
            "normm": np.ascontiguousarray(norm_pad[d]),
            "w1": np.asarray(W1, np.float32).astype(BF16),
            "w2": np.asarray(W2, np.float32).astype(BF16),
            "wmulv": wmulv.astype(BF16),
            "b1d": np.asarray(b1, np.float32),
            "b2d": np.asarray(b2, np.float32),
            "bmulvd": bmulv,
            "iotad": iota_np,
        })

    res = None
    last_err = None
    for attempt in range(4):
        try:
            res = bass_utils.run_bass_kernel_spmd(
                nc, in_maps, core_ids=list(range(NCORES)), trace=TRACE)
            break
        except Exception as e:  # wedged device usually resets on the failed try
            last_err = e
            _time.sleep(2.0 + 2.0 * attempt)
    if res is None:
        raise last_err
    LAST_EXEC_NS = res.exec_time_ns

    mu = np.empty((N, LAT), dtype=np.float32)
    logvar = np.empty((N, LAT), dtype=np.float32)
    for d in range(NCORES):
        sel = dev_of_node == d
        mv = res.results[d]["mulv"]  # [128, LOCAL]
        mu[sel] = mv[:LAT, local_row[sel]].T
        logvar[sel] = mv[LAT:, local_row[sel]].T

    sig_bias = 1.0 / (1.0 + np.exp(-float(dec_bias)))
    adj = np.full((NUM_GRAPHS, MAX_NODES, MAX_NODES), sig_bias, dtype=np.float32)
    for g in range(NUM_GRAPHS):
        d, s = g // GPD, g % GPD
        c = int(counts[g])
        adj[g, :c, :c] = res.results[d]["adj"][s, :c, :c]

    mask = np.arange(MAX_NODES)[None, :] < counts[:, None]
    return adj, mu, logvar, mask
